# revision 36
# baseline (speedup 1.0000x reference)
"""Causal linear attention (B=2, H=8, T=2048, D=64) on 8 Trainium2 NeuronCores.

Sharding: the 16 (batch, head) pairs are split 2-per-core (pure data/head
parallelism; the per-(b,h) recurrence is independent so no collectives).

Per (b,h) the kernel runs a chunked scan over T in chunks of C=128:
  out_chunk = tril(Qp Kp^T) @ [V|1]  +  Qp @ S ,   S += Kp^T @ [V|1]
where Qp/Kp = elu(.)+1 feature maps, and the appended ones-column of V
produces the normalizer z in column D of the output accumulation.
Matmuls run in bf16 with fp32 PSUM accumulation; the running state S is
accumulated in fp32 PSUM (a bf16 snapshot is taken each chunk for the
inter-chunk matmul, so rounding does not compound).

Layout trick: Qp/Kp for the two heads are interleaved as [128, n, head, d]
so one 128x128 PE-mode transpose per chunk yields Qp^T/Kp^T for BOTH
heads, head h landing on partitions [64h, 64h+64) — giving each head a
fixed base partition for all its contraction-dim-64 matmuls. Both heads'
running state accumulates in one PSUM bank (partition-disjoint halves,
head 1 via col-tiling tile_position=(0, 64)), so one bf16 snapshot copy
per chunk serves both inter-chunk matmuls.
"""

import sys

sys.path.insert(0, "/opt/trn_rl_repo")

from contextlib import ExitStack

import numpy as np

import concourse.bass as bass
import concourse.bacc as bacc
import concourse.mybir as mybir
import concourse.tile as tile
from concourse.bass_utils import run_bass_kernel_spmd

B, H, T, D = 2, 8, 2048, 64
N_CORES = 8
PAIRS = B * H                  # 16 (batch, head) pairs
PPC = PAIRS // N_CORES         # 2 pairs per core
C = 128                        # chunk (= partition) size
NCH = T // C                   # 16 chunks

F32 = mybir.dt.float32
BF16 = mybir.dt.bfloat16
AF = mybir.ActivationFunctionType
ALU = mybir.AluOpType

_CACHE = {}


def _build():
    nc = bacc.Bacc(None, target_bir_lowering=False)
    q_d = nc.dram_tensor("q", [PPC, T, D], F32, kind="ExternalInput")
    k_d = nc.dram_tensor("k", [PPC, T, D], F32, kind="ExternalInput")
    v_d = nc.dram_tensor("v", [PPC, T, D], F32, kind="ExternalInput")
    o_d = nc.dram_tensor("out", [PPC, T, D], F32, kind="ExternalOutput")

    with ExitStack() as ctx:
        tc = ctx.enter_context(tile.TileContext(nc))
        consts = ctx.enter_context(tc.tile_pool(name="consts", bufs=1))
        loads = ctx.enter_context(tc.tile_pool(name="loads", bufs=1))
        fmp = ctx.enter_context(tc.tile_pool(name="fmp", bufs=2))
        tpose = ctx.enter_context(tc.tile_pool(name="tpose", bufs=6))
        ampool = ctx.enter_context(tc.tile_pool(name="ampool", bufs=3))
        spool = ctx.enter_context(tc.tile_pool(name="spool", bufs=3))
        opool = ctx.enter_context(tc.tile_pool(name="opool", bufs=1))
        finpool = ctx.enter_context(tc.tile_pool(name="finpool", bufs=2))
        ps_a = ctx.enter_context(tc.tile_pool(name="ps_a", bufs=2, space="PSUM"))
        ps_o = ctx.enter_context(tc.tile_pool(name="ps_o", bufs=3, space="PSUM"))
        ps_s = ctx.enter_context(tc.tile_pool(name="ps_s", bufs=1, space="PSUM"))
        ps_t = ctx.enter_context(tc.tile_pool(name="ps_t", bufs=2, space="PSUM"))

        # Constants (NEFF-embedded): bf16 identity for PE transposes and the
        # upper-triangular (s<=t) mask for A^T[s,t].
        import ml_dtypes

        ident_d = nc.inline_tensor(
            np.eye(C, dtype=np.float32).astype(ml_dtypes.bfloat16), name="ident_c"
        )
        mask_d = nc.inline_tensor(
            np.triu(np.ones((C, C), np.float32)), name="mask_c"
        )
        ident = consts.tile([C, C], BF16, tag="ident")
        nc.gpsimd.dma_start(out=ident, in_=ident_d[:, :])
        mask = consts.tile([C, C], F32, tag="mask")
        nc.gpsimd.dma_start(out=mask, in_=mask_d[:, :])

        # Head-interleaved working layouts.
        qp = fmp.tile([C, NCH, PPC, D], BF16, tag="qp", name="qp")
        kp = fmp.tile([C, NCH, PPC, D], BF16, tag="kp", name="kp")
        vb = fmp.tile([C, NCH, PPC, D + 1], BF16, tag="vb", name="vb")
        HALF = NCH // 2
        osb = opool.tile([C, NCH, PPC, D + 1], BF16, tag="osb", name="osb")

        # Loads + feature maps, interleaved per (head, half) so the first
        # chunk's operands arrive as early as possible.
        qfs, kfs = {}, {}
        for h in range(PPC):
            qfs[h] = loads.tile([C, NCH, D], F32, tag=f"qf{h}", name=f"qf{h}")
            kfs[h] = loads.tile([C, NCH, D], F32, tag=f"kf{h}", name=f"kf{h}")
        for h in range(PPC):
            nc.vector.memset(vb[:, :, h, D : D + 1], 1.0)
        # Progressive load segments: small first so chunk 0's operands (both
        # heads!) land ASAP, then the bulk (emitted later = lower priority,
        # so the bulk feature map doesn't compete with the first chunks).
        def emit_loads(s0, s1):
            rows = slice(s0, s1)
            trows = slice(s0 * C, s1 * C)
            for h in range(PPC):
                nc.sync.dma_start(
                    out=qfs[h][:, rows, :],
                    in_=q_d[h][trows].rearrange("(n p) d -> p n d", p=C),
                )
                nc.sync.dma_start(
                    out=kfs[h][:, rows, :],
                    in_=k_d[h][trows].rearrange("(n p) d -> p n d", p=C),
                )
                # SWDGE cast-DMA fp32 -> bf16 into the strided V' layout.
                nc.gpsimd.dma_start(
                    out=vb[:, rows, h, 0:D],
                    in_=v_d[h][trows].rearrange("(n p) d -> p n d", p=C),
                )

        def emit_fm(s0, s1):
            rows = slice(s0, s1)
            for h in range(PPC):
                # feature map: elu(x)+1 == max(min(exp(x), 1), x+1)
                for src_, dst in ((qfs[h], qp), (kfs[h], kp)):
                    nseg = s1 - s0
                    e = fmp.tile([C, HALF, D], BF16, tag="e", name="e", bufs=4)
                    nc.scalar.activation(
                        out=e[:, :nseg, :], in_=src_[:, rows, :], func=AF.Exp
                    )
                    a = fmp.tile([C, HALF, D], BF16, tag="a", name="a", bufs=4)
                    nc.vector.tensor_scalar_add(
                        out=a[:, :nseg, :], in0=src_[:, rows, :], scalar1=1.0
                    )
                    nc.vector.scalar_tensor_tensor(
                        out=dst[:, rows, h, :],
                        in0=e[:, :nseg, :],
                        scalar=1.0,
                        in1=a[:, :nseg, :],
                        op0=ALU.min,
                        op1=ALU.max,
                    )

        for s0, s1 in ((0, 2), (2, 4), (4, 8)):
            emit_loads(s0, s1)
            emit_fm(s0, s1)
        emit_loads(8, 16)


        # Running state in PSUM: head h accumulates on partitions [64h, 64h+64)
        # of a single shared bank.
        s_psum = ps_s.tile([C, 512], F32, tag="s", name="s_psum")
        s_prev = None

        for n in range(NCH):
            if n == 2:
                # bulk feature map, deliberately after the first two chunks
                emit_fm(8, 12)
                emit_fm(12, 16)

            # PE-mode transposes, one per tensor per chunk; both heads ride
            # along ([128t, (2h x 64d)] -> partitions (h,d), free t).
            # q and k share one PSUM bank; a single op evacuates both.
            tT = ps_t.tile([C, 2, C], BF16, tag="tT", name="tT")
            nc.tensor.transpose(tT[:, 0, :], qp[:, n, :, :], ident)
            nc.tensor.transpose(tT[:, 1, :], kp[:, n, :, :], ident)
            qkT = tpose.tile([C, 2, C], BF16, tag="qkT", name="qkT")
            nc.scalar.activation(out=qkT, in_=tT, func=AF.Copy)

            # State updates first: the chunk-n snapshot is consumed by the
            # inter matmul of chunk n+1, so produce it a full iteration early.
            # Both heads accumulate in ONE bank (h on partitions [64h, 64h+64)
            # via col-tiling) -> a single snapshot copy serves both.
            s_new = None
            if n < NCH - 1:
                for h in range(PPC):
                    lo = h * D
                    nc.tensor.matmul(
                        s_psum[lo : lo + D, 0 : D + 1],
                        kp[:, n, h, :],
                        vb[:, n, h, :],
                        start=(n == 0),
                        stop=True,
                        tile_position=(0, lo),
                        skip_group_check=True,
                    )
                s_new = spool.tile([C, D + 1], BF16, tag="sb", name="sb")
                nc.scalar.activation(
                    out=s_new, in_=s_psum[:, 0 : D + 1], func=AF.Copy
                )

            # Per-head A^T and O (separate banks: heads overlap in the
            # partition dim, and a matmul's start=True clears has_written for
            # its partitions' whole bank).
            for h in range(PPC):
                lo, hi = h * D, (h + 1) * D
                at = ps_a.tile([C, C], F32, tag="at", name="at")
                nc.tensor.matmul(
                    at, qkT[lo:hi, 1, :], qkT[lo:hi, 0, :], start=True, stop=True
                )
                am = ampool.tile([C, C], BF16, tag="am", name="am")
                nc.vector.tensor_mul(am, at, mask)

                op_ = ps_o.tile([C, D + 1], F32, tag="op", name="op")
                nc.tensor.matmul(
                    op_, am, vb[:, n, h, :], start=True, stop=(n == 0)
                )
                if n > 0:
                    nc.tensor.matmul(
                        op_, qkT[lo:hi, 0, :], s_prev[lo:hi, :],
                        start=False, stop=True,
                    )
                if (n + h) % 2 == 0:
                    nc.scalar.activation(out=osb[:, n, h, :], in_=op_, func=AF.Copy)
                else:
                    nc.vector.tensor_copy(out=osb[:, n, h, :], in_=op_)

            if s_new is not None:
                s_prev = s_new

            # normalize + store finished rows (both heads in one pass):
            # first half in one batch, then quarters / per-chunk at the tail.
            batch = None
            if n == HALF - 1:
                batch = slice(0, HALF)
            elif n == 11:
                batch = slice(8, 12)
            elif n >= 12 and n % 2 == 1:
                batch = slice(n - 1, n + 1)
            if batch is not None:
                nb = batch.stop - batch.start
                trows = slice(batch.start * C, batch.stop * C)
                rz = finpool.tile([C, HALF, PPC], F32, tag="rz", name="rz", bufs=4)
                nc.vector.reciprocal(out=rz[:, :nb, :], in_=osb[:, batch, :, D])
                fin = finpool.tile(
                    [C, HALF, PPC, D], F32, tag="fin", name="fin", bufs=4
                )
                rz_b = bass.AP(
                    tensor=rz.tensor,
                    offset=rz.offset,
                    ap=[rz.ap[0], [rz.ap[1][0], nb], rz.ap[2], [0, D]],
                )
                nc.vector.tensor_tensor(
                    out=fin[:, :nb, :, :],
                    in0=osb[:, batch, :, 0:D],
                    in1=rz_b,
                    op=ALU.mult,
                )
                for h in range(PPC):
                    nc.sync.dma_start(
                        out=o_d[h][trows].rearrange("(n p) d -> p n d", p=C),
                        in_=fin[:, :nb, h, :],
                    )

    nc.compile()
    return nc


def _get_program():
    if "nc" not in _CACHE:
        _CACHE["nc"] = _build()
    return _CACHE["nc"]


def run_sharded(q, k, v, trace=False, **kwargs):
    """Run on 8 cores; returns (full_output, BassKernelResults)."""
    nc = _get_program()
    qs = np.ascontiguousarray(np.asarray(q, np.float32).reshape(PAIRS, T, D))
    ks = np.ascontiguousarray(np.asarray(k, np.float32).reshape(PAIRS, T, D))
    vs = np.ascontiguousarray(np.asarray(v, np.float32).reshape(PAIRS, T, D))
    in_maps = [
        {
            "q": qs[i * PPC : (i + 1) * PPC],
            "k": ks[i * PPC : (i + 1) * PPC],
            "v": vs[i * PPC : (i + 1) * PPC],
        }
        for i in range(N_CORES)
    ]
    res = run_bass_kernel_spmd(
        nc, in_maps, core_ids=list(range(N_CORES)), trace=trace, **kwargs
    )
    out = np.concatenate([res.results[i]["out"] for i in range(N_CORES)], axis=0)
    return out.reshape(B, H, T, D).astype(np.float32), res


def kernel(q, k, v):
    out, _ = run_sharded(q, k, v)
    return out
